# revision 31
# baseline (speedup 1.0000x reference)
"""DeepSeekMoE expert-parallel kernel (V7).

Routing on host: gate logits + top-2 + renormalized weights in numpy
fp32 (top-2 margins are ~37x above fp32 matmul noise, so the selection
is exact vs the jax reference). Tokens are compacted into single-expert
banks: every core runs the same static program over NB banks (bank i
has static size s_i, 16-slot granularity; an expert may span several
banks on different cores). Bank sizes AND the expert->bank assignment
are optimized at runtime against the actual per-expert counts with a
small DP, and the program is compiled for those sizes -- per-core work
lands within ~1% of the perfect-balance bound (vs +12% for the naive
one-expert-per-core capacity layout).

Device per bank: h = gelu(x@W1+b1) [I-tiles x slots], yT = (h@W2)^T
[H-tiles x slots]; both matmuls keep slots on the free axis (W2 is the
stationary operand in FFN2), which is what frees bank sizing from the
128-slot psum-partition granularity. bf16 matmuls, fp32 psum
accumulate. Weight sets are double-buffered in SBUF (bufs=2 pool):
bank i+2's weights stream in while bank i computes. Host applies b2
and the gate weight during the weighted scatter-add combine.

All device inputs are host-packed into exact SBUF images so every DMA
is contiguous per partition; weights stream in consumption order; FFN1
runs one sub-chunk ahead of FFN2.
"""
import os
import sys

sys.path.insert(0, "/opt/trn_rl_repo")

import functools
import itertools
import math

import numpy as np
import ml_dtypes

import concourse.bacc as bacc
import concourse.bass as bass  # noqa: F401
import concourse.mybir as mybir
import concourse.tile as tile
from concourse import bass_utils

B, S, H, E, I = 4, 2048, 1024, 8, 2048
T = B * S
NCORE = 8
P = 128
KH = H // P    # 8
KI = I // P    # 16
F32 = mybir.dt.float32
BF16 = mybir.dt.bfloat16
AF = mybir.ActivationFunctionType

# W1 it-tile chunk boundaries for the FIRST weight set: fine-grained
# early so FFN1 on the first sub-chunk streams behind the DMA.
W1_CHUNKS = [(0, 1), (1, 2), (2, 4), (4, 7), (7, 11), (11, KI)]
W2_CHUNKS = [(0, 2), (2, 5), (5, KH)]   # ht chunks, consumption order

LAST_EXEC_NS = None
LAST_RESULT = None


def _install_ntff_shim():
    try:
        import antenv.axon_hooks  # noqa: F401
        return
    except Exception:
        pass
    try:
        import types

        if "/root/.axon_site" not in sys.path:
            sys.path.insert(0, "/root/.axon_site")
        from trn_agent_boot.trn_boot import _ntff_profile_via_ctypes

        hook = _ntff_profile_via_ctypes("/opt/axon/libaxon_pjrt.so")
        mod = types.ModuleType("antenv.axon_hooks")
        mod.get_axon_ntff_profile_hook = lambda: hook
        sys.modules["antenv.axon_hooks"] = mod
    except Exception:
        pass


def _split_subs(n, ascending):
    """Split a bank of n slots into sub-chunks <=512, preferring >=256
    (narrow matmuls fall under the instruction issue floor)."""
    subs = []
    rem = n
    while rem > 512:
        if rem >= 768:
            subs.append(512)
            rem -= 512
        else:
            subs.append(rem - 256)
            rem = 256
    subs.append(rem)
    return sorted(subs) if ascending else sorted(subs, reverse=True)


def _emit_ffn1(nc, pools, w1_sb, b1_sb, x_sb, csz, off):
    hpool, ypool, h_ps, y_ps = pools
    h_sb = hpool.tile([P, KI, 512], BF16, tag="h", name=f"h{off}")
    for it in range(KI):
        ph = h_ps.tile([P, csz], F32, tag="hps", name=f"ph{off}_{it}")
        for k in range(KH):
            nc.tensor.matmul(
                ph[:],
                lhsT=w1_sb[:, it, k, :],
                rhs=x_sb[:, k, 0:csz],
                start=(k == 0),
                stop=(k == KH - 1),
            )
        nc.scalar.activation(
            h_sb[:, it, 0:csz], ph[:], AF.Gelu,
            bias=b1_sb[:, it : it + 1],
        )
    return h_sb


def _emit_ffn2(nc, pools, w2_sb, h_sb, csz, off, yc):
    """Transposed FFN2: psum [128 H-rows, csz slots] per ht; slots stay
    on the free axis so bank sizes need no 128 alignment."""
    hpool, ypool, h_ps, y_ps = pools
    for ht in range(KH):
        py = y_ps.tile([P, csz], F32, tag="yps", name=f"py{off}_{ht}")
        for it in range(KI):
            nc.tensor.matmul(
                py[:],
                lhsT=w2_sb[:, ht, it, :],
                rhs=h_sb[:, it, 0:csz],
                start=(it == 0),
                stop=(it == KI - 1),
            )
        y_sb = ypool.tile([P, 512], BF16, tag="y", name=f"y{off}_{ht}")
        nc.vector.tensor_copy(y_sb[:, 0:csz], py[:])
        nc.sync.dma_start(
            out=yc[ht * P : (ht + 1) * P, off : off + csz],
            in_=y_sb[:, 0:csz],
        )


def _build(bank_subs):
    """bank_subs: list (one entry per bank) of sub-chunk size lists.
    Each bank has its own host-supplied weight set; weight SBUF tiles
    are double-buffered so bank i+2's set streams during bank i."""
    nc = bacc.Bacc(None, target_bir_lowering=False, num_devices=NCORE)

    nb = len(bank_subs)
    cap = sum(sum(s) for s in bank_subs)
    xt = nc.dram_tensor("xt", (P, KH * cap), BF16, kind="ExternalInput")
    yc = nc.dram_tensor("yc", (H, cap), BF16, kind="ExternalOutput")
    w1_r, w2_r, b1_r = [], [], []
    for i in range(nb):
        w1 = nc.dram_tensor(f"w1_{i}", (P, KI * KH * P), BF16,
                            kind="ExternalInput")
        w2 = nc.dram_tensor(f"w2_{i}", (P, KH * KI * P), BF16,
                            kind="ExternalInput")
        b1 = nc.dram_tensor(f"b1_{i}", (P, KI), F32, kind="ExternalInput")
        w1_r.append(w1.rearrange("p (i k j) -> p i k j", i=KI, k=KH))
        w2_r.append(w2.rearrange("p (h i j) -> p h i j", h=KH, i=KI))
        b1_r.append(b1)

    # flat sub list with bank index and slot offset
    flat = []
    off = 0
    for bi, subs in enumerate(bank_subs):
        for csz in subs:
            flat.append((off, csz, bi))
            off += csz

    with tile.TileContext(nc) as tc:
        with (
            tc.tile_pool(name="wpool", bufs=2) as wpool,
            tc.tile_pool(name="xpool", bufs=3) as xpool,
            tc.tile_pool(name="hpool", bufs=3) as hpool,
            tc.tile_pool(name="ypool", bufs=3) as ypool,
            tc.tile_pool(name="h_ps", bufs=3, space="PSUM") as h_ps,
            tc.tile_pool(name="y_ps", bufs=3, space="PSUM") as y_ps,
            tc.tile_pool(name="warm_ps", bufs=1, space="PSUM") as warm_ps,
        ):
            # PE warmup: the tensor engine needs ~3us of continuous
            # execution to reach max clock (2.4GHz); real matmuls can't
            # start until weights/x arrive (~12us in). Fill the DMA
            # prefix with dummy matmuls so the ramp happens off the
            # critical path and real work starts at full clock.
            warm = wpool.tile([1, P], BF16, name="warm")
            nc.vector.memset(warm[:], 0.0)
            wps = warm_ps.tile([P, P], F32, name="wps")
            for _ in range(180):
                nc.tensor.matmul(
                    wps[:], lhsT=warm[:], rhs=warm[:], start=True, stop=True
                )

            x_tiles = []
            for off, csz, bi in flat:
                x_sb = xpool.tile([P, KH, 512], BF16, tag="x", name=f"x{off}")
                x_tiles.append((off, csz, x_sb))

            def x_dma(i):
                off, csz, x_sb = x_tiles[i]
                nc.sync.dma_start(
                    out=x_sb[:, :, 0:csz],
                    in_=xt[:, KH * off : KH * (off + csz)].rearrange(
                        "p (k c) -> p k c", k=KH
                    ),
                )

            # ---- weight set 0 + early x, in consumption order ----
            w1_sb, b1_sb, w2_sb = [], [], []
            w1_sb.append(wpool.tile([P, KI, KH, P], BF16, tag="w1",
                                    name="w1_0"))
            lo, hi = W1_CHUNKS[0]
            nc.sync.dma_start(
                out=w1_sb[0][:, lo:hi, :, :], in_=w1_r[0][:, lo:hi, :, :]
            )
            x_dma(0)
            b1_sb.append(wpool.tile([P, KI], F32, tag="b1", name="b1_0"))
            nc.sync.dma_start(out=b1_sb[0][:], in_=b1_r[0][:])
            for lo, hi in W1_CHUNKS[1:]:
                nc.sync.dma_start(
                    out=w1_sb[0][:, lo:hi, :, :], in_=w1_r[0][:, lo:hi, :, :]
                )
            # x blocks 0..2 can load immediately (xpool bufs=3)
            for i in range(1, min(3, len(flat))):
                x_dma(i)
            w2_sb.append(wpool.tile([P, KH, KI, P], BF16, tag="w2",
                                    name="w2_0"))
            for lo, hi in W2_CHUNKS:
                nc.sync.dma_start(
                    out=w2_sb[0][:, lo:hi, :, :], in_=w2_r[0][:, lo:hi, :, :]
                )
            # ---- later weight sets (pool-recycled), each interleaved
            # with its bank's x blocks, in consumption-deadline order
            nsub = [len(s) for s in bank_subs]
            for j in range(3, nsub[0]):
                x_dma(j)
            for i in range(1, nb):
                w1_sb.append(wpool.tile([P, KI, KH, P], BF16, tag="w1",
                                        name=f"w1_{i}"))
                nc.sync.dma_start(out=w1_sb[i][:], in_=w1_r[i][:])
                b1_sb.append(wpool.tile([P, KI], F32, tag="b1",
                                        name=f"b1_{i}"))
                nc.sync.dma_start(out=b1_sb[i][:], in_=b1_r[i][:])
                for j in range(sum(nsub[:i]), sum(nsub[: i + 1])):
                    if j >= 3:
                        x_dma(j)
                w2_sb.append(wpool.tile([P, KH, KI, P], BF16, tag="w2",
                                        name=f"w2_{i}"))
                nc.sync.dma_start(out=w2_sb[i][:], in_=w2_r[i][:])

            pools = (hpool, ypool, h_ps, y_ps)

            # software pipeline: FFN1 one sub-chunk ahead of FFN2
            h_tiles = [None] * len(flat)
            for i, (off, csz, x_sb) in enumerate(x_tiles):
                bi = flat[i][2]
                h_tiles[i] = _emit_ffn1(
                    nc, pools, w1_sb[bi], b1_sb[bi], x_sb, csz, off
                )
                if i >= 1:
                    offp, cszp, _ = x_tiles[i - 1]
                    _emit_ffn2(
                        nc, pools, w2_sb[flat[i - 1][2]], h_tiles[i - 1],
                        cszp, offp, yc,
                    )
            off, csz, _ = x_tiles[-1]
            _emit_ffn2(nc, pools, w2_sb[flat[-1][2]], h_tiles[-1],
                       csz, off, yc)

    nc.compile()
    return nc


_NC_CACHE = {}


def _get_nc(bank_subs):
    key = tuple(tuple(s) for s in bank_subs)
    if key not in _NC_CACHE:
        _NC_CACHE[key] = _build([list(s) for s in bank_subs])
    return _NC_CACHE[key]


def _r16(v):
    return ((v + 15) // 16) * 16


def _solve_banks(counts, nb):
    """Find bank sizes (nb distinct size classes, NCORE banks each;
    each expert gets exactly nb banks) minimizing per-core total, with
    the expert->pattern assignment. Returns (sizes, assign_patterns)
    or None; assign_patterns[idx] is the size-class multiset for the
    idx-th largest expert."""
    cs = sorted(counts, reverse=True)
    if len(cs) != NCORE:
        return None
    pats = list(itertools.combinations_with_replacement(range(nb), nb))

    def solve(sizes):
        @functools.lru_cache(maxsize=None)
        def rec(idx, avail):
            if idx == NCORE:
                return () if all(a == 0 for a in avail) else None
            for pat in pats:
                if sum(sizes[i] for i in pat) < cs[idx]:
                    continue
                av = list(avail)
                ok = True
                for i in pat:
                    av[i] -= 1
                    if av[i] < 0:
                        ok = False
                        break
                if not ok:
                    continue
                sub = rec(idx + 1, tuple(av))
                if sub is not None:
                    return (pat,) + sub
            return None
        return rec(0, tuple([NCORE] * nb))

    base = sum(cs) // NCORE
    lo = max(256, _r16(base // nb - 208))
    hi = _r16(base // nb + 304)
    grid = sorted(range(lo, hi, 16), reverse=True)
    best = None
    for sizes in itertools.combinations_with_replacement(grid, nb):
        tot = sum(sizes)
        if tot < base or (best and tot >= best[0]):
            continue
        pats_assign = solve(sizes)
        if pats_assign is not None:
            best = (tot, sizes, pats_assign)
    if best is None:
        return None
    return best[1], best[2]


def _pack_x(x_cols_bf, subs):
    """Pack [H, ncols] bf16 into the SBUF image [P, KH*cap] with
    per-sub contiguous blocks."""
    cap = sum(subs)
    img = np.zeros((P, KH * cap), dtype=ml_dtypes.bfloat16)
    off = 0
    for csz in subs:
        blk = np.zeros((H, csz), dtype=ml_dtypes.bfloat16)
        n = min(max(x_cols_bf.shape[1] - off, 0), csz)
        if n > 0:
            blk[:, :n] = x_cols_bf[:, off : off + n]
        img[:, KH * off : KH * (off + csz)] = (
            blk.reshape(KH, P, csz).transpose(1, 0, 2).reshape(P, KH * csz)
        )
        off += csz
    return img


def _pack_wset(W1e, W2e, b1e, i):
    w1 = np.asarray(W1e, dtype=np.float32).astype(ml_dtypes.bfloat16)
    # [H, I] -> [P, KI, KH, P]: img[p, it, k, j] = w1[k*128+p, it*128+j]
    w1i = (
        w1.reshape(KH, P, KI, P).transpose(1, 2, 0, 3).reshape(P, KI * KH * P)
    )
    w2 = np.asarray(W2e, dtype=np.float32).astype(ml_dtypes.bfloat16)
    # [I, H] -> [P, KH, KI, P]: img[p, ht, it, j] = w2[it*128+p, ht*128+j]
    w2i = (
        w2.reshape(KI, P, KH, P).transpose(1, 2, 0, 3).reshape(P, KH * KI * P)
    )
    b1i = np.ascontiguousarray(
        np.asarray(b1e, dtype=np.float32).reshape(KI, P).T
    )
    return {
        f"w1_{i}": np.ascontiguousarray(w1i),
        f"w2_{i}": np.ascontiguousarray(w2i),
        f"b1_{i}": b1i,
    }


def kernel(hidden_states, Wg, W1, b1, W2, b2):
    global LAST_EXEC_NS, LAST_RESULT
    if os.environ.get("BASS_TRACE"):
        _install_ntff_shim()

    x = np.asarray(hidden_states, dtype=np.float32).reshape(T, H)
    Wg = np.asarray(Wg, dtype=np.float32)
    W1 = np.asarray(W1, dtype=np.float32)
    W2 = np.asarray(W2, dtype=np.float32)
    b1 = np.asarray(b1, dtype=np.float32)
    b2 = np.asarray(b2, dtype=np.float32)

    # ---- host routing (fp32 gate; exact vs jax) ----
    logits = x @ Wg                                        # [T, E] fp32
    order = np.argsort(-logits, axis=1, kind="stable")     # jax tie-break
    i0, i1 = order[:, 0], order[:, 1]
    rows = np.arange(T)
    l0 = logits[rows, i0].astype(np.float64)
    l1 = logits[rows, i1].astype(np.float64)
    g0 = (1.0 / (1.0 + np.exp(l1 - l0))).astype(np.float32)
    g1 = (1.0 - g0).astype(np.float32)

    x_bf = x.astype(ml_dtypes.bfloat16)

    sel_e = []
    gate_e = []
    for e in range(E):
        sel = np.where((i0 == e) | (i1 == e))[0]
        sel_e.append(sel)
        gate_e.append(np.where(i0[sel] == e, g0[sel], g1[sel]))
    counts = [len(s) for s in sel_e]

    sol = _solve_banks(counts, 3) or _solve_banks(counts, 2)

    if sol is not None:
        sizes, pats = sol
        nb = len(sizes)
        # bank i of a core covers slots [bank_off[i], bank_off[i]+sizes[i])
        bank_off = [sum(sizes[:i]) for i in range(nb)]
        bank_subs = [
            _split_subs(sizes[i], ascending=(i == 0)) for i in range(nb)
        ]
        cap = sum(sizes)

        # materialize (core, bank) slots per size class
        stacks = [[(c, i) for c in range(NCORE)] for i in range(nb)]
        eorder = sorted(range(E), key=lambda e: -counts[e])
        core_banks = {c: [] for c in range(NCORE)}
        used = {}
        ok = True
        for idx, e in enumerate(eorder):
            pos = 0
            for cls in pats[idx]:
                if not stacks[cls]:
                    ok = False
                    break
                core, bi = stacks[cls].pop()
                take = max(0, min(sizes[bi], counts[e] - pos))
                if take > 0:
                    core_banks[core].append(
                        (bank_off[bi], bi, e, sel_e[e][pos : pos + take],
                         gate_e[e][pos : pos + take])
                    )
                    used[(core, bi)] = e
                pos += take
            if not ok or pos < counts[e]:
                ok = False
                break

        if ok:
            in_maps = []
            for core in range(NCORE):
                xcols = np.zeros((H, cap), dtype=ml_dtypes.bfloat16)
                for off, bi, e, toks, _ in core_banks[core]:
                    xcols[:, off : off + len(toks)] = x_bf[toks].T
                m = {
                    "xt": _pack_x(
                        xcols, [c for s in bank_subs for c in s]
                    )
                }
                for bi in range(nb):
                    e = used.get((core, bi), 0)
                    m.update(_pack_wset(W1[e], W2[e], b1[e], bi))
                in_maps.append(m)

            nc = _get_nc(bank_subs)
            res = bass_utils.run_bass_kernel_spmd(
                nc, in_maps, core_ids=list(range(NCORE))
            )
            LAST_EXEC_NS = res.exec_time_ns
            LAST_RESULT = res

            out = np.zeros((T, H), dtype=np.float32)
            for core in range(NCORE):
                yt = res.results[core]["yc"]          # [H, cap] bf16
                for off, bi, e, toks, g in core_banks[core]:
                    y = (
                        yt[:, off : off + len(toks)].T.astype(np.float32)
                        + b2[e]
                    )
                    out[toks] += g[:, None] * y
            return (
                np.ascontiguousarray(out).reshape(B, S, H).astype(np.float32)
            )

    # ---- fallback: one expert per core, sized for the largest ----
    capf = _r16(min(max(counts), T))
    subs_f = _split_subs(capf, ascending=True)
    in_maps = []
    for e in range(E):
        sel = sel_e[e][:capf]
        m = {"xt": _pack_x(x_bf[sel].T, subs_f)}
        m.update(_pack_wset(W1[e], W2[e], b1[e], 0))
        in_maps.append(m)
    nc = _get_nc([subs_f])
    res = bass_utils.run_bass_kernel_spmd(
        nc, in_maps, core_ids=list(range(NCORE))
    )
    LAST_EXEC_NS = res.exec_time_ns
    LAST_RESULT = res
    out = np.zeros((T, H), dtype=np.float32)
    for e in range(E):
        sel = sel_e[e][:capf]
        n = len(sel)
        y = res.results[e]["yc"][:, :n].T.astype(np.float32) + b2[e]
        out[sel] += gate_e[e][:n, None] * y
    return np.ascontiguousarray(out).reshape(B, S, H).astype(np.float32)


# revision 32
# speedup vs baseline: 1.0567x; 1.0567x over previous
"""DeepSeekMoE expert-parallel kernel (V7).

Routing on host: gate logits + top-2 + renormalized weights in numpy
fp32 (top-2 margins are ~37x above fp32 matmul noise, so the selection
is exact vs the jax reference). Tokens are compacted into single-expert
banks: every core runs the same static program over NB banks (bank i
has static size s_i, 16-slot granularity; an expert may span several
banks on different cores). Bank sizes AND the expert->bank assignment
are optimized at runtime against the actual per-expert counts with a
small DP, and the program is compiled for those sizes -- per-core work
lands within ~1% of the perfect-balance bound (vs +12% for the naive
one-expert-per-core capacity layout).

Device per bank: h = gelu(x@W1+b1) [I-tiles x slots], yT = (h@W2)^T
[H-tiles x slots]; both matmuls keep slots on the free axis (W2 is the
stationary operand in FFN2), which is what frees bank sizing from the
128-slot psum-partition granularity. bf16 matmuls, fp32 psum
accumulate. Weight sets are double-buffered in SBUF (bufs=2 pool):
bank i+2's weights stream in while bank i computes. Host applies b2
and the gate weight during the weighted scatter-add combine.

All device inputs are host-packed into exact SBUF images so every DMA
is contiguous per partition; weights stream in consumption order; FFN1
runs one sub-chunk ahead of FFN2.
"""
import os
import sys

sys.path.insert(0, "/opt/trn_rl_repo")

import functools
import itertools
import math

import numpy as np
import ml_dtypes

import concourse.bacc as bacc
import concourse.bass as bass  # noqa: F401
import concourse.mybir as mybir
import concourse.tile as tile
from concourse import bass_utils

B, S, H, E, I = 4, 2048, 1024, 8, 2048
T = B * S
NCORE = 8
P = 128
KH = H // P    # 8
KI = I // P    # 16
F32 = mybir.dt.float32
BF16 = mybir.dt.bfloat16
AF = mybir.ActivationFunctionType

# W1 it-tile chunk boundaries for the FIRST weight set: fine-grained
# early so FFN1 on the first sub-chunk streams behind the DMA.
W1_CHUNKS = [(0, 1), (1, 2), (2, 4), (4, 7), (7, 11), (11, KI)]
W2_CHUNKS = [(0, 2), (2, 5), (5, KH)]   # ht chunks, consumption order

LAST_EXEC_NS = None
LAST_RESULT = None


def _install_ntff_shim():
    try:
        import antenv.axon_hooks  # noqa: F401
        return
    except Exception:
        pass
    try:
        import types

        if "/root/.axon_site" not in sys.path:
            sys.path.insert(0, "/root/.axon_site")
        from trn_agent_boot.trn_boot import _ntff_profile_via_ctypes

        hook = _ntff_profile_via_ctypes("/opt/axon/libaxon_pjrt.so")
        mod = types.ModuleType("antenv.axon_hooks")
        mod.get_axon_ntff_profile_hook = lambda: hook
        sys.modules["antenv.axon_hooks"] = mod
    except Exception:
        pass


def _split_subs(n, ascending):
    """Split a bank of n slots into sub-chunks <=512, preferring >=256
    (narrow matmuls fall under the instruction issue floor)."""
    subs = []
    rem = n
    while rem > 512:
        if rem >= 768:
            subs.append(512)
            rem -= 512
        else:
            subs.append(rem - 256)
            rem = 256
    subs.append(rem)
    return sorted(subs) if ascending else sorted(subs, reverse=True)


def _emit_ffn1(nc, pools, w1_sb, b1_sb, x_sb, csz, off):
    hpool, ypool, h_ps, y_ps = pools
    h_sb = hpool.tile([P, KI, 512], BF16, tag="h", name=f"h{off}")
    for it in range(KI):
        ph = h_ps.tile([P, csz], F32, tag="hps", name=f"ph{off}_{it}")
        for k in range(KH):
            nc.tensor.matmul(
                ph[:],
                lhsT=w1_sb[:, it, k, :],
                rhs=x_sb[:, k, 0:csz],
                start=(k == 0),
                stop=(k == KH - 1),
            )
        nc.scalar.activation(
            h_sb[:, it, 0:csz], ph[:], AF.Gelu,
            bias=b1_sb[:, it : it + 1],
        )
    return h_sb


def _emit_ffn2(nc, pools, w2_sb, h_sb, csz, off, yc):
    """Transposed FFN2: psum [128 H-rows, csz slots] per ht; slots stay
    on the free axis so bank sizes need no 128 alignment."""
    hpool, ypool, h_ps, y_ps = pools
    for ht in range(KH):
        py = y_ps.tile([P, csz], F32, tag="yps", name=f"py{off}_{ht}")
        for it in range(KI):
            nc.tensor.matmul(
                py[:],
                lhsT=w2_sb[:, ht, it, :],
                rhs=h_sb[:, it, 0:csz],
                start=(it == 0),
                stop=(it == KI - 1),
            )
        y_sb = ypool.tile([P, 512], BF16, tag="y", name=f"y{off}_{ht}")
        nc.vector.tensor_copy(y_sb[:, 0:csz], py[:])
        nc.sync.dma_start(
            out=yc[ht * P : (ht + 1) * P, off : off + csz],
            in_=y_sb[:, 0:csz],
        )


def _build(bank_subs):
    """bank_subs: list (one entry per bank) of sub-chunk size lists.
    Each bank has its own host-supplied weight set; weight SBUF tiles
    are double-buffered so bank i+2's set streams during bank i."""
    nc = bacc.Bacc(None, target_bir_lowering=False, num_devices=NCORE)

    nb = len(bank_subs)
    cap = sum(sum(s) for s in bank_subs)
    xt = nc.dram_tensor("xt", (P, KH * cap), BF16, kind="ExternalInput")
    yc = nc.dram_tensor("yc", (H, cap), BF16, kind="ExternalOutput")
    w1_r, w2_r, b1_r = [], [], []
    for i in range(nb):
        w1 = nc.dram_tensor(f"w1_{i}", (P, KI * KH * P), BF16,
                            kind="ExternalInput")
        w2 = nc.dram_tensor(f"w2_{i}", (P, KH * KI * P), BF16,
                            kind="ExternalInput")
        b1 = nc.dram_tensor(f"b1_{i}", (P, KI), F32, kind="ExternalInput")
        w1_r.append(w1.rearrange("p (i k j) -> p i k j", i=KI, k=KH))
        w2_r.append(w2.rearrange("p (h i j) -> p h i j", h=KH, i=KI))
        b1_r.append(b1)

    # flat sub list with bank index and slot offset
    flat = []
    off = 0
    for bi, subs in enumerate(bank_subs):
        for csz in subs:
            flat.append((off, csz, bi))
            off += csz

    with tile.TileContext(nc) as tc:
        with (
            tc.tile_pool(name="wpool", bufs=2) as wpool,
            tc.tile_pool(name="xpool", bufs=3) as xpool,
            tc.tile_pool(name="hpool", bufs=3) as hpool,
            tc.tile_pool(name="ypool", bufs=3) as ypool,
            tc.tile_pool(name="h_ps", bufs=3, space="PSUM") as h_ps,
            tc.tile_pool(name="y_ps", bufs=3, space="PSUM") as y_ps,
        ):
            x_tiles = []
            for off, csz, bi in flat:
                x_sb = xpool.tile([P, KH, 512], BF16, tag="x", name=f"x{off}")
                x_tiles.append((off, csz, x_sb))

            def x_dma(i):
                off, csz, x_sb = x_tiles[i]
                nc.sync.dma_start(
                    out=x_sb[:, :, 0:csz],
                    in_=xt[:, KH * off : KH * (off + csz)].rearrange(
                        "p (k c) -> p k c", k=KH
                    ),
                )

            # ---- weight set 0 + early x, in consumption order ----
            w1_sb, b1_sb, w2_sb = [], [], []
            w1_sb.append(wpool.tile([P, KI, KH, P], BF16, tag="w1",
                                    name="w1_0"))
            lo, hi = W1_CHUNKS[0]
            nc.sync.dma_start(
                out=w1_sb[0][:, lo:hi, :, :], in_=w1_r[0][:, lo:hi, :, :]
            )
            x_dma(0)
            b1_sb.append(wpool.tile([P, KI], F32, tag="b1", name="b1_0"))
            nc.sync.dma_start(out=b1_sb[0][:], in_=b1_r[0][:])
            for lo, hi in W1_CHUNKS[1:]:
                nc.sync.dma_start(
                    out=w1_sb[0][:, lo:hi, :, :], in_=w1_r[0][:, lo:hi, :, :]
                )
            # x blocks 0..2 can load immediately (xpool bufs=3)
            for i in range(1, min(3, len(flat))):
                x_dma(i)
            w2_sb.append(wpool.tile([P, KH, KI, P], BF16, tag="w2",
                                    name="w2_0"))
            for lo, hi in W2_CHUNKS:
                nc.sync.dma_start(
                    out=w2_sb[0][:, lo:hi, :, :], in_=w2_r[0][:, lo:hi, :, :]
                )
            # ---- later weight sets (pool-recycled), each interleaved
            # with its bank's x blocks, in consumption-deadline order
            nsub = [len(s) for s in bank_subs]
            for j in range(3, nsub[0]):
                x_dma(j)
            for i in range(1, nb):
                w1_sb.append(wpool.tile([P, KI, KH, P], BF16, tag="w1",
                                        name=f"w1_{i}"))
                nc.sync.dma_start(out=w1_sb[i][:], in_=w1_r[i][:])
                b1_sb.append(wpool.tile([P, KI], F32, tag="b1",
                                        name=f"b1_{i}"))
                nc.sync.dma_start(out=b1_sb[i][:], in_=b1_r[i][:])
                for j in range(sum(nsub[:i]), sum(nsub[: i + 1])):
                    if j >= 3:
                        x_dma(j)
                w2_sb.append(wpool.tile([P, KH, KI, P], BF16, tag="w2",
                                        name=f"w2_{i}"))
                nc.sync.dma_start(out=w2_sb[i][:], in_=w2_r[i][:])

            pools = (hpool, ypool, h_ps, y_ps)

            # software pipeline: FFN1 one sub-chunk ahead of FFN2
            h_tiles = [None] * len(flat)
            for i, (off, csz, x_sb) in enumerate(x_tiles):
                bi = flat[i][2]
                h_tiles[i] = _emit_ffn1(
                    nc, pools, w1_sb[bi], b1_sb[bi], x_sb, csz, off
                )
                if i >= 1:
                    offp, cszp, _ = x_tiles[i - 1]
                    _emit_ffn2(
                        nc, pools, w2_sb[flat[i - 1][2]], h_tiles[i - 1],
                        cszp, offp, yc,
                    )
            off, csz, _ = x_tiles[-1]
            _emit_ffn2(nc, pools, w2_sb[flat[-1][2]], h_tiles[-1],
                       csz, off, yc)

    nc.compile()
    return nc


_NC_CACHE = {}


def _get_nc(bank_subs):
    key = tuple(tuple(s) for s in bank_subs)
    if key not in _NC_CACHE:
        _NC_CACHE[key] = _build([list(s) for s in bank_subs])
    return _NC_CACHE[key]


def _r16(v):
    return ((v + 15) // 16) * 16


def _solve_banks(counts, nb):
    """Find bank sizes (nb distinct size classes, NCORE banks each;
    each expert gets exactly nb banks) minimizing per-core total, with
    the expert->pattern assignment. Returns (sizes, assign_patterns)
    or None; assign_patterns[idx] is the size-class multiset for the
    idx-th largest expert."""
    cs = sorted(counts, reverse=True)
    if len(cs) != NCORE:
        return None
    pats = list(itertools.combinations_with_replacement(range(nb), nb))

    def solve(sizes):
        @functools.lru_cache(maxsize=None)
        def rec(idx, avail):
            if idx == NCORE:
                return () if all(a == 0 for a in avail) else None
            for pat in pats:
                if sum(sizes[i] for i in pat) < cs[idx]:
                    continue
                av = list(avail)
                ok = True
                for i in pat:
                    av[i] -= 1
                    if av[i] < 0:
                        ok = False
                        break
                if not ok:
                    continue
                sub = rec(idx + 1, tuple(av))
                if sub is not None:
                    return (pat,) + sub
            return None
        return rec(0, tuple([NCORE] * nb))

    base = sum(cs) // NCORE
    lo = max(256, _r16(base // nb - 208))
    hi = _r16(base // nb + 304)
    grid = sorted(range(lo, hi, 16), reverse=True)
    best = None
    for sizes in itertools.combinations_with_replacement(grid, nb):
        tot = sum(sizes)
        if tot < base or (best and tot >= best[0]):
            continue
        pats_assign = solve(sizes)
        if pats_assign is not None:
            best = (tot, sizes, pats_assign)
    if best is None:
        return None
    return best[1], best[2]


def _pack_x(x_cols_bf, subs):
    """Pack [H, ncols] bf16 into the SBUF image [P, KH*cap] with
    per-sub contiguous blocks."""
    cap = sum(subs)
    img = np.zeros((P, KH * cap), dtype=ml_dtypes.bfloat16)
    off = 0
    for csz in subs:
        blk = np.zeros((H, csz), dtype=ml_dtypes.bfloat16)
        n = min(max(x_cols_bf.shape[1] - off, 0), csz)
        if n > 0:
            blk[:, :n] = x_cols_bf[:, off : off + n]
        img[:, KH * off : KH * (off + csz)] = (
            blk.reshape(KH, P, csz).transpose(1, 0, 2).reshape(P, KH * csz)
        )
        off += csz
    return img


def _pack_wset(W1e, W2e, b1e, i):
    w1 = np.asarray(W1e, dtype=np.float32).astype(ml_dtypes.bfloat16)
    # [H, I] -> [P, KI, KH, P]: img[p, it, k, j] = w1[k*128+p, it*128+j]
    w1i = (
        w1.reshape(KH, P, KI, P).transpose(1, 2, 0, 3).reshape(P, KI * KH * P)
    )
    w2 = np.asarray(W2e, dtype=np.float32).astype(ml_dtypes.bfloat16)
    # [I, H] -> [P, KH, KI, P]: img[p, ht, it, j] = w2[it*128+p, ht*128+j]
    w2i = (
        w2.reshape(KI, P, KH, P).transpose(1, 2, 0, 3).reshape(P, KH * KI * P)
    )
    b1i = np.ascontiguousarray(
        np.asarray(b1e, dtype=np.float32).reshape(KI, P).T
    )
    return {
        f"w1_{i}": np.ascontiguousarray(w1i),
        f"w2_{i}": np.ascontiguousarray(w2i),
        f"b1_{i}": b1i,
    }


def kernel(hidden_states, Wg, W1, b1, W2, b2):
    global LAST_EXEC_NS, LAST_RESULT
    if os.environ.get("BASS_TRACE"):
        _install_ntff_shim()

    x = np.asarray(hidden_states, dtype=np.float32).reshape(T, H)
    Wg = np.asarray(Wg, dtype=np.float32)
    W1 = np.asarray(W1, dtype=np.float32)
    W2 = np.asarray(W2, dtype=np.float32)
    b1 = np.asarray(b1, dtype=np.float32)
    b2 = np.asarray(b2, dtype=np.float32)

    # ---- host routing (fp32 gate; exact vs jax) ----
    logits = x @ Wg                                        # [T, E] fp32
    order = np.argsort(-logits, axis=1, kind="stable")     # jax tie-break
    i0, i1 = order[:, 0], order[:, 1]
    rows = np.arange(T)
    l0 = logits[rows, i0].astype(np.float64)
    l1 = logits[rows, i1].astype(np.float64)
    g0 = (1.0 / (1.0 + np.exp(l1 - l0))).astype(np.float32)
    g1 = (1.0 - g0).astype(np.float32)

    x_bf = x.astype(ml_dtypes.bfloat16)

    sel_e = []
    gate_e = []
    for e in range(E):
        sel = np.where((i0 == e) | (i1 == e))[0]
        sel_e.append(sel)
        gate_e.append(np.where(i0[sel] == e, g0[sel], g1[sel]))
    counts = [len(s) for s in sel_e]

    sol = _solve_banks(counts, 3) or _solve_banks(counts, 2)

    if sol is not None:
        sizes, pats = sol
        nb = len(sizes)
        # bank i of a core covers slots [bank_off[i], bank_off[i]+sizes[i])
        bank_off = [sum(sizes[:i]) for i in range(nb)]
        bank_subs = [
            _split_subs(sizes[i], ascending=(i == 0)) for i in range(nb)
        ]
        cap = sum(sizes)

        # materialize (core, bank) slots per size class
        stacks = [[(c, i) for c in range(NCORE)] for i in range(nb)]
        eorder = sorted(range(E), key=lambda e: -counts[e])
        core_banks = {c: [] for c in range(NCORE)}
        used = {}
        ok = True
        for idx, e in enumerate(eorder):
            pos = 0
            for cls in pats[idx]:
                if not stacks[cls]:
                    ok = False
                    break
                core, bi = stacks[cls].pop()
                take = max(0, min(sizes[bi], counts[e] - pos))
                if take > 0:
                    core_banks[core].append(
                        (bank_off[bi], bi, e, sel_e[e][pos : pos + take],
                         gate_e[e][pos : pos + take])
                    )
                    used[(core, bi)] = e
                pos += take
            if not ok or pos < counts[e]:
                ok = False
                break

        if ok:
            in_maps = []
            for core in range(NCORE):
                xcols = np.zeros((H, cap), dtype=ml_dtypes.bfloat16)
                for off, bi, e, toks, _ in core_banks[core]:
                    xcols[:, off : off + len(toks)] = x_bf[toks].T
                m = {
                    "xt": _pack_x(
                        xcols, [c for s in bank_subs for c in s]
                    )
                }
                for bi in range(nb):
                    e = used.get((core, bi), 0)
                    m.update(_pack_wset(W1[e], W2[e], b1[e], bi))
                in_maps.append(m)

            nc = _get_nc(bank_subs)
            res = bass_utils.run_bass_kernel_spmd(
                nc, in_maps, core_ids=list(range(NCORE))
            )
            LAST_EXEC_NS = res.exec_time_ns
            LAST_RESULT = res

            out = np.zeros((T, H), dtype=np.float32)
            for core in range(NCORE):
                yt = res.results[core]["yc"]          # [H, cap] bf16
                for off, bi, e, toks, g in core_banks[core]:
                    y = (
                        yt[:, off : off + len(toks)].T.astype(np.float32)
                        + b2[e]
                    )
                    out[toks] += g[:, None] * y
            return (
                np.ascontiguousarray(out).reshape(B, S, H).astype(np.float32)
            )

    # ---- fallback: one expert per core, sized for the largest ----
    capf = _r16(min(max(counts), T))
    subs_f = _split_subs(capf, ascending=True)
    in_maps = []
    for e in range(E):
        sel = sel_e[e][:capf]
        m = {"xt": _pack_x(x_bf[sel].T, subs_f)}
        m.update(_pack_wset(W1[e], W2[e], b1[e], 0))
        in_maps.append(m)
    nc = _get_nc([subs_f])
    res = bass_utils.run_bass_kernel_spmd(
        nc, in_maps, core_ids=list(range(NCORE))
    )
    LAST_EXEC_NS = res.exec_time_ns
    LAST_RESULT = res
    out = np.zeros((T, H), dtype=np.float32)
    for e in range(E):
        sel = sel_e[e][:capf]
        n = len(sel)
        y = res.results[e]["yc"][:, :n].T.astype(np.float32) + b2[e]
        out[sel] += gate_e[e][:n, None] * y
    return np.ascontiguousarray(out).reshape(B, S, H).astype(np.float32)


# revision 33
# speedup vs baseline: 1.0569x; 1.0003x over previous
"""DeepSeekMoE expert-parallel kernel (V7).

Routing on host: gate logits + top-2 + renormalized weights in numpy
fp32 (top-2 margins are ~37x above fp32 matmul noise, so the selection
is exact vs the jax reference). Tokens are compacted into single-expert
banks: every core runs the same static program over NB banks (bank i
has static size s_i, 16-slot granularity; an expert may span several
banks on different cores). Bank sizes AND the expert->bank assignment
are optimized at runtime against the actual per-expert counts with a
small DP, and the program is compiled for those sizes -- per-core work
lands within ~1% of the perfect-balance bound (vs +12% for the naive
one-expert-per-core capacity layout).

Device per bank: h = gelu(x@W1+b1) [I-tiles x slots], yT = (h@W2)^T
[H-tiles x slots]; both matmuls keep slots on the free axis (W2 is the
stationary operand in FFN2), which is what frees bank sizing from the
128-slot psum-partition granularity. bf16 matmuls, fp32 psum
accumulate. Weight sets are double-buffered in SBUF (bufs=2 pool):
bank i+2's weights stream in while bank i computes. Host applies b2
and the gate weight during the weighted scatter-add combine.

All device inputs are host-packed into exact SBUF images so every DMA
is contiguous per partition; weights stream in consumption order; FFN1
runs one sub-chunk ahead of FFN2.
"""
import os
import sys

sys.path.insert(0, "/opt/trn_rl_repo")

import functools
import itertools
import math

import numpy as np
import ml_dtypes

import concourse.bacc as bacc
import concourse.bass as bass  # noqa: F401
import concourse.mybir as mybir
import concourse.tile as tile
from concourse import bass_utils

B, S, H, E, I = 4, 2048, 1024, 8, 2048
T = B * S
NCORE = 8
P = 128
KH = H // P    # 8
KI = I // P    # 16
F32 = mybir.dt.float32
BF16 = mybir.dt.bfloat16
AF = mybir.ActivationFunctionType

# W1 it-tile chunk boundaries for the FIRST weight set: fine-grained
# early so FFN1 on the first sub-chunk streams behind the DMA.
W1_CHUNKS = [(0, 1), (1, 2), (2, 4), (4, 7), (7, 11), (11, KI)]
W2_CHUNKS = [(0, 2), (2, 5), (5, KH)]   # ht chunks, consumption order

LAST_EXEC_NS = None
LAST_RESULT = None


def _install_ntff_shim():
    try:
        import antenv.axon_hooks  # noqa: F401
        return
    except Exception:
        pass
    try:
        import types

        if "/root/.axon_site" not in sys.path:
            sys.path.insert(0, "/root/.axon_site")
        from trn_agent_boot.trn_boot import _ntff_profile_via_ctypes

        hook = _ntff_profile_via_ctypes("/opt/axon/libaxon_pjrt.so")
        mod = types.ModuleType("antenv.axon_hooks")
        mod.get_axon_ntff_profile_hook = lambda: hook
        sys.modules["antenv.axon_hooks"] = mod
    except Exception:
        pass


def _split_subs(n, ascending):
    """Split a bank of n slots into sub-chunks <=512, preferring >=256
    (narrow matmuls fall under the instruction issue floor)."""
    subs = []
    rem = n
    while rem > 512:
        if rem >= 768:
            subs.append(512)
            rem -= 512
        else:
            subs.append(rem - 256)
            rem = 256
    subs.append(rem)
    return sorted(subs) if ascending else sorted(subs, reverse=True)


def _emit_ffn1(nc, pools, w1_sb, b1_sb, x_sb, csz, off):
    hpool, ypool, h_ps, y_ps = pools
    h_sb = hpool.tile([P, KI, 512], BF16, tag="h", name=f"h{off}")
    for it in range(KI):
        ph = h_ps.tile([P, csz], F32, tag="hps", name=f"ph{off}_{it}")
        for k in range(KH):
            nc.tensor.matmul(
                ph[:],
                lhsT=w1_sb[:, it, k, :],
                rhs=x_sb[:, k, 0:csz],
                start=(k == 0),
                stop=(k == KH - 1),
            )
        nc.scalar.activation(
            h_sb[:, it, 0:csz], ph[:], AF.Gelu,
            bias=b1_sb[:, it : it + 1],
        )
    return h_sb


def _emit_ffn2(nc, pools, w2_sb, h_sb, csz, off, yc):
    """Transposed FFN2: psum [128 H-rows, csz slots] per ht; slots stay
    on the free axis so bank sizes need no 128 alignment."""
    hpool, ypool, h_ps, y_ps = pools
    for ht in range(KH):
        py = y_ps.tile([P, csz], F32, tag="yps", name=f"py{off}_{ht}")
        for it in range(KI):
            nc.tensor.matmul(
                py[:],
                lhsT=w2_sb[:, ht, it, :],
                rhs=h_sb[:, it, 0:csz],
                start=(it == 0),
                stop=(it == KI - 1),
            )
        y_sb = ypool.tile([P, 512], BF16, tag="y", name=f"y{off}_{ht}")
        nc.vector.tensor_copy(y_sb[:, 0:csz], py[:])
        nc.sync.dma_start(
            out=yc[ht * P : (ht + 1) * P, off : off + csz],
            in_=y_sb[:, 0:csz],
        )


def _build(bank_subs):
    """bank_subs: list (one entry per bank) of sub-chunk size lists.
    Each bank has its own host-supplied weight set; weight SBUF tiles
    are double-buffered so bank i+2's set streams during bank i."""
    nc = bacc.Bacc(None, target_bir_lowering=False, num_devices=NCORE)

    nb = len(bank_subs)
    cap = sum(sum(s) for s in bank_subs)
    xt = nc.dram_tensor("xt", (P, KH * cap), BF16, kind="ExternalInput")
    yc = nc.dram_tensor("yc", (H, cap), BF16, kind="ExternalOutput")
    w1_r, w2_r, b1_r = [], [], []
    for i in range(nb):
        w1 = nc.dram_tensor(f"w1_{i}", (P, KI * KH * P), BF16,
                            kind="ExternalInput")
        w2 = nc.dram_tensor(f"w2_{i}", (P, KH * KI * P), BF16,
                            kind="ExternalInput")
        b1 = nc.dram_tensor(f"b1_{i}", (P, KI), F32, kind="ExternalInput")
        w1_r.append(w1.rearrange("p (i k j) -> p i k j", i=KI, k=KH))
        w2_r.append(w2.rearrange("p (h i j) -> p h i j", h=KH, i=KI))
        b1_r.append(b1)

    # flat sub list with bank index and slot offset
    flat = []
    off = 0
    for bi, subs in enumerate(bank_subs):
        for csz in subs:
            flat.append((off, csz, bi))
            off += csz

    with tile.TileContext(nc) as tc:
        with (
            tc.tile_pool(name="wpool", bufs=2) as wpool,
            tc.tile_pool(name="xpool", bufs=3) as xpool,
            tc.tile_pool(name="hpool", bufs=3) as hpool,
            tc.tile_pool(name="ypool", bufs=3) as ypool,
            tc.tile_pool(name="h_ps", bufs=3, space="PSUM") as h_ps,
            tc.tile_pool(name="y_ps", bufs=3, space="PSUM") as y_ps,
        ):
            x_tiles = []
            for off, csz, bi in flat:
                # exact-size tile: DMA is contiguous on both sides
                x_sb = xpool.tile([P, KH, csz], BF16, tag="x", name=f"x{off}")
                x_tiles.append((off, csz, x_sb))

            def x_dma(i):
                off, csz, x_sb = x_tiles[i]
                nc.sync.dma_start(
                    out=x_sb[:, :, 0:csz],
                    in_=xt[:, KH * off : KH * (off + csz)].rearrange(
                        "p (k c) -> p k c", k=KH
                    ),
                )

            # ---- weight set 0 + early x, in consumption order ----
            w1_sb, b1_sb, w2_sb = [], [], []
            w1_sb.append(wpool.tile([P, KI, KH, P], BF16, tag="w1",
                                    name="w1_0"))
            lo, hi = W1_CHUNKS[0]
            nc.sync.dma_start(
                out=w1_sb[0][:, lo:hi, :, :], in_=w1_r[0][:, lo:hi, :, :]
            )
            x_dma(0)
            b1_sb.append(wpool.tile([P, KI], F32, tag="b1", name="b1_0"))
            nc.sync.dma_start(out=b1_sb[0][:], in_=b1_r[0][:])
            for lo, hi in W1_CHUNKS[1:]:
                nc.sync.dma_start(
                    out=w1_sb[0][:, lo:hi, :, :], in_=w1_r[0][:, lo:hi, :, :]
                )
            # x blocks 0..2 can load immediately (xpool bufs=3)
            for i in range(1, min(3, len(flat))):
                x_dma(i)
            w2_sb.append(wpool.tile([P, KH, KI, P], BF16, tag="w2",
                                    name="w2_0"))
            for lo, hi in W2_CHUNKS:
                nc.sync.dma_start(
                    out=w2_sb[0][:, lo:hi, :, :], in_=w2_r[0][:, lo:hi, :, :]
                )
            # ---- later weight sets (pool-recycled), each interleaved
            # with its bank's x blocks, in consumption-deadline order
            nsub = [len(s) for s in bank_subs]
            for j in range(3, nsub[0]):
                x_dma(j)
            for i in range(1, nb):
                w1_sb.append(wpool.tile([P, KI, KH, P], BF16, tag="w1",
                                        name=f"w1_{i}"))
                nc.sync.dma_start(out=w1_sb[i][:], in_=w1_r[i][:])
                b1_sb.append(wpool.tile([P, KI], F32, tag="b1",
                                        name=f"b1_{i}"))
                nc.sync.dma_start(out=b1_sb[i][:], in_=b1_r[i][:])
                for j in range(sum(nsub[:i]), sum(nsub[: i + 1])):
                    if j >= 3:
                        x_dma(j)
                w2_sb.append(wpool.tile([P, KH, KI, P], BF16, tag="w2",
                                        name=f"w2_{i}"))
                nc.sync.dma_start(out=w2_sb[i][:], in_=w2_r[i][:])

            pools = (hpool, ypool, h_ps, y_ps)

            # software pipeline: FFN1 one sub-chunk ahead of FFN2
            h_tiles = [None] * len(flat)
            for i, (off, csz, x_sb) in enumerate(x_tiles):
                bi = flat[i][2]
                h_tiles[i] = _emit_ffn1(
                    nc, pools, w1_sb[bi], b1_sb[bi], x_sb, csz, off
                )
                if i >= 1:
                    offp, cszp, _ = x_tiles[i - 1]
                    _emit_ffn2(
                        nc, pools, w2_sb[flat[i - 1][2]], h_tiles[i - 1],
                        cszp, offp, yc,
                    )
            off, csz, _ = x_tiles[-1]
            _emit_ffn2(nc, pools, w2_sb[flat[-1][2]], h_tiles[-1],
                       csz, off, yc)

    nc.compile()
    return nc


_NC_CACHE = {}


def _get_nc(bank_subs):
    key = tuple(tuple(s) for s in bank_subs)
    if key not in _NC_CACHE:
        _NC_CACHE[key] = _build([list(s) for s in bank_subs])
    return _NC_CACHE[key]


def _r16(v):
    return ((v + 15) // 16) * 16


def _solve_banks(counts, nb):
    """Find bank sizes (nb distinct size classes, NCORE banks each;
    each expert gets exactly nb banks) minimizing per-core total, with
    the expert->pattern assignment. Returns (sizes, assign_patterns)
    or None; assign_patterns[idx] is the size-class multiset for the
    idx-th largest expert."""
    cs = sorted(counts, reverse=True)
    if len(cs) != NCORE:
        return None
    pats = list(itertools.combinations_with_replacement(range(nb), nb))

    def solve(sizes):
        @functools.lru_cache(maxsize=None)
        def rec(idx, avail):
            if idx == NCORE:
                return () if all(a == 0 for a in avail) else None
            for pat in pats:
                if sum(sizes[i] for i in pat) < cs[idx]:
                    continue
                av = list(avail)
                ok = True
                for i in pat:
                    av[i] -= 1
                    if av[i] < 0:
                        ok = False
                        break
                if not ok:
                    continue
                sub = rec(idx + 1, tuple(av))
                if sub is not None:
                    return (pat,) + sub
            return None
        return rec(0, tuple([NCORE] * nb))

    base = sum(cs) // NCORE
    lo = max(256, _r16(base // nb - 208))
    hi = _r16(base // nb + 304)
    grid = sorted(range(lo, hi, 16), reverse=True)
    best = None
    for sizes in itertools.combinations_with_replacement(grid, nb):
        tot = sum(sizes)
        if tot < base or (best and tot >= best[0]):
            continue
        pats_assign = solve(sizes)
        if pats_assign is not None:
            best = (tot, sizes, pats_assign)
    if best is None:
        return None
    return best[1], best[2]


def _pack_x(x_cols_bf, subs):
    """Pack [H, ncols] bf16 into the SBUF image [P, KH*cap] with
    per-sub contiguous blocks."""
    cap = sum(subs)
    img = np.zeros((P, KH * cap), dtype=ml_dtypes.bfloat16)
    off = 0
    for csz in subs:
        blk = np.zeros((H, csz), dtype=ml_dtypes.bfloat16)
        n = min(max(x_cols_bf.shape[1] - off, 0), csz)
        if n > 0:
            blk[:, :n] = x_cols_bf[:, off : off + n]
        img[:, KH * off : KH * (off + csz)] = (
            blk.reshape(KH, P, csz).transpose(1, 0, 2).reshape(P, KH * csz)
        )
        off += csz
    return img


def _pack_wset(W1e, W2e, b1e, i):
    w1 = np.asarray(W1e, dtype=np.float32).astype(ml_dtypes.bfloat16)
    # [H, I] -> [P, KI, KH, P]: img[p, it, k, j] = w1[k*128+p, it*128+j]
    w1i = (
        w1.reshape(KH, P, KI, P).transpose(1, 2, 0, 3).reshape(P, KI * KH * P)
    )
    w2 = np.asarray(W2e, dtype=np.float32).astype(ml_dtypes.bfloat16)
    # [I, H] -> [P, KH, KI, P]: img[p, ht, it, j] = w2[it*128+p, ht*128+j]
    w2i = (
        w2.reshape(KI, P, KH, P).transpose(1, 2, 0, 3).reshape(P, KH * KI * P)
    )
    b1i = np.ascontiguousarray(
        np.asarray(b1e, dtype=np.float32).reshape(KI, P).T
    )
    return {
        f"w1_{i}": np.ascontiguousarray(w1i),
        f"w2_{i}": np.ascontiguousarray(w2i),
        f"b1_{i}": b1i,
    }


def kernel(hidden_states, Wg, W1, b1, W2, b2):
    global LAST_EXEC_NS, LAST_RESULT
    if os.environ.get("BASS_TRACE"):
        _install_ntff_shim()

    x = np.asarray(hidden_states, dtype=np.float32).reshape(T, H)
    Wg = np.asarray(Wg, dtype=np.float32)
    W1 = np.asarray(W1, dtype=np.float32)
    W2 = np.asarray(W2, dtype=np.float32)
    b1 = np.asarray(b1, dtype=np.float32)
    b2 = np.asarray(b2, dtype=np.float32)

    # ---- host routing (fp32 gate; exact vs jax) ----
    logits = x @ Wg                                        # [T, E] fp32
    order = np.argsort(-logits, axis=1, kind="stable")     # jax tie-break
    i0, i1 = order[:, 0], order[:, 1]
    rows = np.arange(T)
    l0 = logits[rows, i0].astype(np.float64)
    l1 = logits[rows, i1].astype(np.float64)
    g0 = (1.0 / (1.0 + np.exp(l1 - l0))).astype(np.float32)
    g1 = (1.0 - g0).astype(np.float32)

    x_bf = x.astype(ml_dtypes.bfloat16)

    sel_e = []
    gate_e = []
    for e in range(E):
        sel = np.where((i0 == e) | (i1 == e))[0]
        sel_e.append(sel)
        gate_e.append(np.where(i0[sel] == e, g0[sel], g1[sel]))
    counts = [len(s) for s in sel_e]

    sol = _solve_banks(counts, 3) or _solve_banks(counts, 2)

    if sol is not None:
        sizes, pats = sol
        nb = len(sizes)
        # bank i of a core covers slots [bank_off[i], bank_off[i]+sizes[i])
        bank_off = [sum(sizes[:i]) for i in range(nb)]
        bank_subs = [
            _split_subs(sizes[i], ascending=(i == 0)) for i in range(nb)
        ]
        cap = sum(sizes)

        # materialize (core, bank) slots per size class
        stacks = [[(c, i) for c in range(NCORE)] for i in range(nb)]
        eorder = sorted(range(E), key=lambda e: -counts[e])
        core_banks = {c: [] for c in range(NCORE)}
        used = {}
        ok = True
        for idx, e in enumerate(eorder):
            pos = 0
            for cls in pats[idx]:
                if not stacks[cls]:
                    ok = False
                    break
                core, bi = stacks[cls].pop()
                take = max(0, min(sizes[bi], counts[e] - pos))
                if take > 0:
                    core_banks[core].append(
                        (bank_off[bi], bi, e, sel_e[e][pos : pos + take],
                         gate_e[e][pos : pos + take])
                    )
                    used[(core, bi)] = e
                pos += take
            if not ok or pos < counts[e]:
                ok = False
                break

        if ok:
            in_maps = []
            for core in range(NCORE):
                xcols = np.zeros((H, cap), dtype=ml_dtypes.bfloat16)
                for off, bi, e, toks, _ in core_banks[core]:
                    xcols[:, off : off + len(toks)] = x_bf[toks].T
                m = {
                    "xt": _pack_x(
                        xcols, [c for s in bank_subs for c in s]
                    )
                }
                for bi in range(nb):
                    e = used.get((core, bi), 0)
                    m.update(_pack_wset(W1[e], W2[e], b1[e], bi))
                in_maps.append(m)

            nc = _get_nc(bank_subs)
            res = bass_utils.run_bass_kernel_spmd(
                nc, in_maps, core_ids=list(range(NCORE))
            )
            LAST_EXEC_NS = res.exec_time_ns
            LAST_RESULT = res

            out = np.zeros((T, H), dtype=np.float32)
            for core in range(NCORE):
                yt = res.results[core]["yc"]          # [H, cap] bf16
                for off, bi, e, toks, g in core_banks[core]:
                    y = (
                        yt[:, off : off + len(toks)].T.astype(np.float32)
                        + b2[e]
                    )
                    out[toks] += g[:, None] * y
            return (
                np.ascontiguousarray(out).reshape(B, S, H).astype(np.float32)
            )

    # ---- fallback: one expert per core, sized for the largest ----
    capf = _r16(min(max(counts), T))
    subs_f = _split_subs(capf, ascending=True)
    in_maps = []
    for e in range(E):
        sel = sel_e[e][:capf]
        m = {"xt": _pack_x(x_bf[sel].T, subs_f)}
        m.update(_pack_wset(W1[e], W2[e], b1[e], 0))
        in_maps.append(m)
    nc = _get_nc([subs_f])
    res = bass_utils.run_bass_kernel_spmd(
        nc, in_maps, core_ids=list(range(NCORE))
    )
    LAST_EXEC_NS = res.exec_time_ns
    LAST_RESULT = res
    out = np.zeros((T, H), dtype=np.float32)
    for e in range(E):
        sel = sel_e[e][:capf]
        n = len(sel)
        y = res.results[e]["yc"][:, :n].T.astype(np.float32) + b2[e]
        out[sel] += gate_e[e][:n, None] * y
    return np.ascontiguousarray(out).reshape(B, S, H).astype(np.float32)
